# revision 5
# baseline (speedup 1.0000x reference)
"""V5: routed kernel, resident weights, DVE bias, sorted expert slots.

Data-parallel over 8 cores (1024 rows each), weights replicated.  Host sorts
each core's rows by expert per module type.  Per (core, module-type) the
experts are RELABELED into slots ordered by ascending group size, so the
static slot capacities CAPS (multiples of 32, ascending, from the actual
data) waste less padding than a uniform C: slot 0 usually fits in 2 chunks
of <=128 instead of 3, cutting one 512-cycle PE pass per swapped layer.
Weights/biases are permuted to slot order per core on the host.

- All weight/bias constants load ONCE (outside the repeat loop) and stay
  resident in SBUF; steady-state DMA is only xe/tbl, 3 permutation
  round-trips and the output.
- Swapped layers: row-bias is added by DVE during PSUM evacuation
  (scalar_tensor_tensor), ReLU applied in-place by Pool in SBUF.
- xe and Wf are bf16.
"""

import numpy as np
import ml_dtypes
from contextlib import ExitStack

import concourse.bass as bass
import concourse.bacc as bacc
import concourse.tile as tile
import concourse.mybir as mybir
from concourse import bass_utils

F32 = mybir.dt.float32
BF16 = mybir.dt.bfloat16
I32 = mybir.dt.int32
RELU = mybir.ActivationFunctionType.Relu
COPY = mybir.ActivationFunctionType.Copy

B = 8192
NCORES = 8
BC = B // NCORES
FEAT = 32
M = 4
H = 512
OUT = 8
P = 128
KBIG = [4, 8, 8, 8]


def _chunks(caps):
    """Static chunk split of each slot group: [(s, g0, r)] with r<=128."""
    out = []
    off0 = 0
    for s, cap in enumerate(caps):
        off = 0
        while off < cap:
            r = min(P, cap - off)
            out.append((s, off0 + off, r))
            off += r
        off0 += cap
    return out


def _offs(caps):
    o, acc = [], 0
    for c in caps:
        o.append(acc)
        acc += c
    return o, acc


def _emit_consts(nc, tc, ctx, d):
    """One-time loads: weights, biases. Stays resident across iterations."""
    consts = ctx.enter_context(tc.tile_pool(name="consts", bufs=1))
    cst = {}
    wf_t = []
    for j in range(4):
        t = consts.tile([P, H], BF16, tag=f"wf{j}", name=f"wf{j}")
        nc.sync.dma_start(t[:], d["Wf"].ap()[j, :, :])
        wf_t.append(t)
    cst["wf"] = wf_t
    w32_t = []
    for s in range(M):
        t = consts.tile([P, 4, OUT], BF16, tag=f"w32_{s}", name=f"w32_{s}")
        nc.sync.dma_start(
            t[:], d["W32"].ap()[s, :, :].rearrange("(a p) o -> p a o", p=P))
        w32_t.append(t)
    cst["w32"] = w32_t
    # big per-slot weight stacks, fully resident: wt[j][s][k] = [128, 512]
    wt = []
    for j in range(4):
        per_s = []
        for s in range(M):
            ks = []
            for k in range(KBIG[j]):
                w = consts.tile([P, H], BF16, tag=f"w{j}_{s}_{k}",
                                name=f"w{j}_{s}_{k}")
                nc.sync.dma_start(
                    w[:], d[f"W{j}1"].ap()[s, k * P:(k + 1) * P, :])
                ks.append(w)
            per_s.append(ks)
        wt.append(per_s)
    cst["wt"] = wt
    bias_sb = consts.tile([P, 8 * 16], F32, tag="bias", name="bias")
    nc.sync.dma_start(bias_sb[:], d["bias"].ap())
    cst["bias"] = bias_sb
    bh = consts.tile([OUT, 4], F32, tag="bh", name="bh")
    nc.sync.dma_start(bh[:], d["bh"].ap())
    cst["bh"] = bh
    # broadcast row-biases for the swapped layers: [(t*4+s)] -> [128, H]
    bbc = consts.tile([P, 12 * H], BF16, tag="bbc", name="bbc")
    nc.sync.dma_start(bbc[:], d["bbc"].ap())
    cst["bbc"] = bbc
    return cst


def _emit_body(nc, tc, ctx, d, caps, cst, pools):
    offs, Bp = _offs(caps)
    Cmax = max(caps)
    chunks = _chunks(caps)
    NCH = len(chunks)
    xep, hp, permp, outp, psp = pools

    wf_t, w32_t, wt = cst["wf"], cst["w32"], cst["wt"]
    bias_sb, bh, bbc = cst["bias"], cst["bh"], cst["bbc"]

    xe_t = []
    for j in range(4):
        t = xep.tile([P, Bp], BF16, tag=f"xe{j}", name=f"xe{j}", bufs=2)
        nc.sync.dma_start(t[:], d["xe"].ap()[j, :, :])
        xe_t.append(t)
    tbl = xep.tile([P, 3 * NCH], I32, tag="tbl", name="tbl", bufs=2)
    nc.sync.dma_start(tbl[:], d["tbl"].ap())

    def bias_ap(layer, hh, s):
        col = layer * 16 + hh * 4 + s
        return bias_sb[:, col:col + 1]

    def scol(t, s):
        return slice(offs[s], offs[s] + caps[s])

    # ---------------- layers ----------------
    def first_layer(j, tag):
        """relu(Wf[j].T @ xe_g[j] + b_j0): 4x [128, Bp] bf16, feature-major."""
        outs = []
        for hpair in range(2):
            ps = [[psp.tile([P, Cmax], F32, tag="pt4", name="pt4")
                   for s in range(M)] for _ in range(2)]
            for hi in range(2):
                hh = hpair * 2 + hi
                for s in range(M):
                    nc.tensor.matmul(
                        ps[hi][s][:, :caps[s]], wf_t[j][:, bass.ts(hh, P)],
                        xe_t[j][:, scol(j, s)], start=True, stop=True)
            for hi in range(2):
                hh = hpair * 2 + hi
                t = hp.tile([P, Bp], BF16, tag=f"{tag}{hh}", name=f"{tag}{hh}")
                for s in range(M):
                    nc.scalar.activation(t[:, scol(j, s)],
                                         ps[hi][s][:, :caps[s]],
                                         RELU, bias=bias_ap(2 * j, hh, s))
                outs.append(t)
        return outs

    def swapped_big(j, z_tiles, t_i):
        """relu(W_j1[slot].T @ z + b), batch-major out -> xsc token tile.

        DVE evacuates PSUM with the row-bias added; Pool applies ReLU
        in-place in SBUF."""
        Kc = KBIG[j]
        xsc = permp.tile([P, NCH, H], BF16, tag="xsc", name="xsc")
        for ch, (s, g0, r) in enumerate(chunks):
            ws = wt[j][s]
            bcol = (t_i * 4 + s) * H
            pb = psp.tile([P, H], F32, tag="pt", name="pt")
            for k in range(Kc):
                nc.tensor.matmul(pb[:r, :], z_tiles[k][:, g0:g0 + r],
                                 ws[k][:],
                                 start=(k == 0), stop=(k == Kc - 1))
            nc.vector.scalar_tensor_tensor(
                xsc[:r, ch, :], pb[:r, :], 0.0, bbc[:r, bcol:bcol + H],
                mybir.AluOpType.bypass, mybir.AluOpType.add)
            nc.gpsimd.tensor_scalar_max(xsc[:r, ch, :], xsc[:r, ch, :], 0.0)
        return xsc

    def transition(t_i, xsc):
        """Scatter chunk tokens into next stage's order; XBAR back."""
        xb = d["xb"][t_i]
        for ch, (s, g0, r) in enumerate(chunks):
            nc.gpsimd.indirect_dma_start(
                xb.ap(),
                bass.IndirectOffsetOnAxis(
                    ap=tbl[:r, t_i * NCH + ch:t_i * NCH + ch + 1], axis=0),
                xsc[:r, ch, :], None)
        zx = permp.tile([P, 4, Bp], BF16, tag="zx", name="zx")
        for k in range(4):
            nc.sync.dma_start(zx[:, k, :], xb.ap()[:, k * P:(k + 1) * P],
                              transpose=True)
        return [zx[:, k, :] for k in range(4)]

    def grouped_big(j, z_tiles, tag):
        """relu(W_j1[slot].T @ z + b): feature-major grouped output."""
        Kc = KBIG[j]
        outs = [hp.tile([P, Bp], BF16, tag=f"{tag}{hh}", name=f"{tag}{hh}")
                for hh in range(4)]
        for s in range(M):
            ws = wt[j][s]
            ps = [psp.tile([P, Cmax], F32, tag="pt4", name="pt4")
                  for hh in range(4)]
            for k in range(Kc):
                for hh in range(4):
                    nc.tensor.matmul(
                        ps[hh][:, :caps[s]], ws[k][:, bass.ts(hh, P)],
                        z_tiles[k][:, scol(j, s)],
                        start=(k == 0), stop=(k == Kc - 1))
            for hh in range(4):
                nc.scalar.activation(outs[hh][:, scol(j, s)],
                                     ps[hh][:, :caps[s]],
                                     RELU, bias=bias_ap(2 * j + 1, hh, s))
        return outs

    # ---------------- network ----------------
    x = first_layer(0, "h")
    xsc = swapped_big(0, x, 0)
    zx = transition(0, xsc)
    h1 = first_layer(1, "g")
    xsc = swapped_big(1, zx + h1, 1)
    zx = transition(1, xsc)
    h2 = first_layer(2, "h")
    xsc = swapped_big(2, zx + h2, 2)
    zx = transition(2, xsc)
    h3 = first_layer(3, "g")
    x4 = grouped_big(3, zx + h3, "x4")

    # head
    ps = [psp.tile([OUT, Cmax], F32, tag="pth", name="pth") for s in range(M)]
    for k in range(4):
        for s in range(M):
            nc.tensor.matmul(ps[s][:, :caps[s]], w32_t[s][:, k, :],
                             x4[k][:, scol(3, s)],
                             start=(k == 0), stop=(k == 3))
    out_t = outp.tile([OUT, Bp], F32, tag="outt", name="outt", bufs=2)
    for s in range(M):
        nc.scalar.activation(out_t[:, scol(3, s)], ps[s][:, :caps[s]], COPY)
        nc.vector.tensor_scalar_add(out_t[:, scol(3, s)],
                                    out_t[:, scol(3, s)], bh[:, s:s + 1])
    nc.sync.dma_start(d["out"].ap(), out_t[:])


def build_program(caps, reps: int = 1):
    _, Bp = _offs(caps)
    NCH = len(_chunks(caps))
    nc = bacc.Bacc("TRN2", target_bir_lowering=False, debug=False,
                   enable_asserts=False)
    d = {}
    d["xe"] = nc.dram_tensor("xe", [4, P, Bp], BF16, kind="ExternalInput")
    d["Wf"] = nc.dram_tensor("Wf", [4, P, H], BF16, kind="ExternalInput")
    d["W01"] = nc.dram_tensor("W01", [M, H, H], BF16, kind="ExternalInput")
    d["W11"] = nc.dram_tensor("W11", [M, 2 * H, H], BF16, kind="ExternalInput")
    d["W21"] = nc.dram_tensor("W21", [M, 2 * H, H], BF16, kind="ExternalInput")
    d["W31"] = nc.dram_tensor("W31", [M, 2 * H, H], BF16, kind="ExternalInput")
    d["W32"] = nc.dram_tensor("W32", [M, H, OUT], BF16, kind="ExternalInput")
    d["bias"] = nc.dram_tensor("bias", [P, 8 * 16], F32, kind="ExternalInput")
    d["bh"] = nc.dram_tensor("bh", [OUT, 4], F32, kind="ExternalInput")
    d["bbc"] = nc.dram_tensor("bbc", [P, 12 * H], BF16, kind="ExternalInput")
    d["tbl"] = nc.dram_tensor("tbl", [P, 3 * NCH], I32, kind="ExternalInput")
    d["out"] = nc.dram_tensor("out", [OUT, Bp], F32, kind="ExternalOutput")
    d["xb"] = [nc.dram_tensor(f"xb{i}", [Bp, H], BF16, kind="Internal")
               for i in range(3)]

    with tile.TileContext(nc) as tc, ExitStack() as ctx:
        cst = _emit_consts(nc, tc, ctx, d)
        xep = ctx.enter_context(tc.tile_pool(name="xein", bufs=2))
        hp = ctx.enter_context(tc.tile_pool(name="hacts", bufs=1))
        permp = ctx.enter_context(tc.tile_pool(name="perm", bufs=1))
        outp = ctx.enter_context(tc.tile_pool(name="outs", bufs=2))
        psp = ctx.enter_context(tc.tile_pool(name="psum", bufs=8,
                                             space="PSUM"))
        pools = (xep, hp, permp, outp, psp)
        if reps == 1:
            _emit_body(nc, tc, ctx, d, caps, cst, pools)
        else:
            with tc.For_i(0, reps, 1):
                _emit_body(nc, tc, ctx, d, caps, cst, pools)
    nc.compile()
    return nc


def prep_inputs(inputs):
    iv = np.asarray(inputs["input_val"], dtype=np.float32)
    feats = iv[:, :4 * FEAT]
    oh = iv[:, 4 * FEAT:4 * FEAT + 16]
    idx = [np.argmax(oh[:, 4 * j:4 * j + 4], axis=1) for j in range(4)]

    # per-(core,j) slot order: experts sorted by ascending group size
    perms = np.empty((NCORES, 4, M), np.int64)   # perms[c,j,slot] = expert
    counts = np.empty((NCORES, 4, M), np.int64)  # slot-ordered group sizes
    for c in range(NCORES):
        rc = slice(c * BC, (c + 1) * BC)
        for j in range(4):
            cnt = np.bincount(idx[j][rc], minlength=M)
            perms[c, j] = np.argsort(cnt, kind="stable")
            counts[c, j] = cnt[perms[c, j]]
    rank_max = counts.reshape(-1, M).max(axis=0)
    caps = tuple(int(((v + 31) // 32) * 32) for v in rank_max)
    offs, Bp = _offs(caps)
    chunks = _chunks(caps)
    NCH = len(chunks)

    tobf = lambda a: np.ascontiguousarray(
        np.asarray(a, np.float32).astype(ml_dtypes.bfloat16))

    Wf_full = np.stack(
        [np.asarray(inputs[f"W{j}_0"], np.float32) for j in range(4)])
    # Wf_full[j] is [M, FEAT, H]
    b0 = np.stack(
        [np.asarray(inputs[f"b{j}_0"], np.float32) for j in range(4)])
    b31 = np.asarray(inputs["b3_1"], np.float32)
    bsw = np.stack([np.asarray(inputs[nm], np.float32)
                    for nm in ("b0_1", "b1_1", "b2_1")])  # [3, M, H]
    Wbig = [np.asarray(inputs[f"W{j}_1"], np.float32) for j in range(4)]
    W32_full = np.asarray(inputs["W3_2"], np.float32)
    bh_full = np.asarray(inputs["b3_2"], np.float32)     # [M, OUT]

    in_maps, meta = [], []
    for c in range(NCORES):
        rc = slice(c * BC, (c + 1) * BC)
        pc = perms[c]                       # [4, M] slot -> expert
        inv = np.empty((4, M), np.int64)    # expert -> slot
        for j in range(4):
            inv[j, pc[j]] = np.arange(M)
        sidx = [inv[j][idx[j][rc]] for j in range(4)]   # slot index per row

        orders, slots, padlists = [], [], []
        for j in range(4):
            ij = sidx[j]
            order = np.full(Bp, -1, np.int64)
            slot = np.empty(BC, np.int64)
            pads = []
            for s in range(M):
                rows = np.nonzero(ij == s)[0]
                order[offs[s]:offs[s] + len(rows)] = rows
                slot[rows] = offs[s] + np.arange(len(rows))
                pads.extend(range(offs[s] + len(rows), offs[s] + caps[s]))
            orders.append(order)
            slots.append(slot)
            padlists.append(np.array(pads, np.int64))

        xe = np.zeros((4, P, Bp), np.float32)
        for j in range(4):
            ij = sidx[j]
            fj = feats[rc, FEAT * j:FEAT * (j + 1)]
            for s in range(M):
                rows = np.nonzero(ij == s)[0]
                xe[j, s * FEAT:(s + 1) * FEAT,
                   offs[s]:offs[s] + len(rows)] = fj[rows].T

        tblv = np.full((P, 3 * NCH), Bp + 7, np.int32)   # default: OOB skip
        for t in range(3):
            jp, jn = t, t + 1
            padmap = {int(g): i for i, g in enumerate(padlists[jp])}
            for ch, (s, g0, r) in enumerate(chunks):
                for p in range(r):
                    g = g0 + p
                    src = orders[jp][g]
                    if src >= 0:
                        tblv[p, t * NCH + ch] = slots[jn][src]
                    else:
                        tblv[p, t * NCH + ch] = padlists[jn][padmap[g]]

        # slot-permuted weights/biases for this core
        Wf_c = np.zeros((4, P, H), np.float32)
        bias = np.zeros((P, 8 * 16), np.float32)
        for j in range(4):
            for s in range(M):
                e = pc[j, s]
                Wf_c[j, s * FEAT:(s + 1) * FEAT] = Wf_full[j, e]
                for hh in range(4):
                    bias[:, 2 * j * 16 + hh * 4 + s] = \
                        b0[j, e, hh * P:(hh + 1) * P]
        for s in range(M):
            e = pc[3, s]
            for hh in range(4):
                bias[:, 7 * 16 + hh * 4 + s] = b31[e, hh * P:(hh + 1) * P]
        bbc = np.zeros((P, 12 * H), np.float32)
        for t in range(3):
            for s in range(M):
                e = pc[t, s]
                bbc[:, (t * 4 + s) * H:(t * 4 + s + 1) * H] = \
                    bsw[t, e][None, :]
        in_maps.append({
            "xe": tobf(xe), "tbl": tblv,
            "Wf": tobf(Wf_c), "bias": bias, "bbc": tobf(bbc),
            "W01": tobf(Wbig[0][pc[0]]), "W11": tobf(Wbig[1][pc[1]]),
            "W21": tobf(Wbig[2][pc[2]]), "W31": tobf(Wbig[3][pc[3]]),
            "W32": tobf(W32_full[pc[3]]),
            "bh": np.ascontiguousarray(bh_full[pc[3]].T),
        })
        meta.append(slots[3])
    return caps, in_maps, meta


_CACHE = {}


def kernel(**inputs):
    caps, in_maps, meta = prep_inputs(inputs)
    if ("nc", caps) not in _CACHE:
        _CACHE[("nc", caps)] = build_program(caps)
    nc = _CACHE[("nc", caps)]
    res = bass_utils.run_bass_kernel_spmd(
        nc, in_maps, core_ids=list(range(NCORES)))
    out = np.empty((B, OUT), np.float32)
    for c in range(NCORES):
        o = res.results[c]["out"]
        out[c * BC:(c + 1) * BC] = o[:, meta[c]].T
    return out


if __name__ == "__main__":
    import sys, jax
    import reference
    cpu = jax.local_devices(backend="cpu")[0]
    with jax.default_device(cpu):
        inputs = {k: np.asarray(v) for k, v in reference.setup_inputs().items()}
        exp = np.asarray(reference.reference(**inputs))
    if len(sys.argv) > 1 and sys.argv[1] == "sim":
        from concourse.bass_interp import CoreSim
        caps, in_maps, meta = prep_inputs(inputs)
        print("caps:", caps)
        nc = build_program(caps)
        sim = CoreSim(nc, trace=True)
        for k, v in in_maps[0].items():
            sim.tensor(k)[:] = v
        sim.simulate()
        o = np.asarray(sim.tensor("out"))
        got0 = o[:, meta[0]].T
        exp0 = exp[:BC]
        err = np.abs(got0 - exp0)
        print(f"sim core0 max abs err: {err.max():.3e}  "
              f"rel: {err.max()/np.abs(exp0).max():.3e}")
    else:
        got = kernel(**inputs)
        err = np.abs(got - exp)
        print(f"max abs err: {err.max():.3e}   "
              f"rel: {err.max()/np.abs(exp).max():.3e}")


# revision 7
# speedup vs baseline: 3.0727x; 3.0727x over previous
"""V5: routed kernel, resident weights, DVE bias, sorted expert slots.

Data-parallel over 8 cores (1024 rows each), weights replicated.  Host sorts
each core's rows by expert per module type.  Per (core, module-type) the
experts are RELABELED into slots ordered by ascending group size, so the
static slot capacities CAPS (multiples of 32, ascending, from the actual
data) waste less padding than a uniform C: slot 0 usually fits in 2 chunks
of <=128 instead of 3, cutting one 512-cycle PE pass per swapped layer.
Weights/biases are permuted to slot order per core on the host.

- All weight/bias constants load ONCE (outside the repeat loop) and stay
  resident in SBUF; steady-state DMA is only xe/tbl, 3 permutation
  round-trips and the output.
- Swapped layers: row-bias is added by DVE during PSUM evacuation
  (scalar_tensor_tensor), ReLU applied in-place by Pool in SBUF.
- xe and Wf are bf16.
"""

import numpy as np
import ml_dtypes
from contextlib import ExitStack

import concourse.bass as bass
import concourse.bacc as bacc
import concourse.tile as tile
import concourse.mybir as mybir
from concourse import bass_utils

F32 = mybir.dt.float32
BF16 = mybir.dt.bfloat16
I32 = mybir.dt.int32
RELU = mybir.ActivationFunctionType.Relu
COPY = mybir.ActivationFunctionType.Copy

B = 8192
NCORES = 8
BC = B // NCORES
FEAT = 32
M = 4
H = 512
OUT = 8
P = 128
KBIG = [4, 8, 8, 8]


def _chunks(caps):
    """Static chunk split of each slot group: [(s, g0, r)] with r<=128."""
    out = []
    off0 = 0
    for s, cap in enumerate(caps):
        off = 0
        while off < cap:
            r = min(P, cap - off)
            out.append((s, off0 + off, r))
            off += r
        off0 += cap
    return out


def _offs(caps):
    o, acc = [], 0
    for c in caps:
        o.append(acc)
        acc += c
    return o, acc


def _emit_consts(nc, tc, ctx, d):
    """One-time loads: weights, biases. Stays resident across iterations."""
    consts = ctx.enter_context(tc.tile_pool(name="consts", bufs=1))
    cst = {}
    wf_t = []
    for j in range(4):
        t = consts.tile([P, H], BF16, tag=f"wf{j}", name=f"wf{j}")
        nc.sync.dma_start(t[:], d["Wf"].ap()[j, :, :])
        wf_t.append(t)
    cst["wf"] = wf_t
    w32_t = []
    for s in range(M):
        t = consts.tile([P, 4, OUT], BF16, tag=f"w32_{s}", name=f"w32_{s}")
        nc.sync.dma_start(
            t[:], d["W32"].ap()[s, :, :].rearrange("(a p) o -> p a o", p=P))
        w32_t.append(t)
    cst["w32"] = w32_t
    # big per-slot weight stacks, fully resident: wt[j][s][k] = [128, 512]
    wt = []
    for j in range(4):
        per_s = []
        for s in range(M):
            ks = []
            for k in range(KBIG[j]):
                w = consts.tile([P, H], BF16, tag=f"w{j}_{s}_{k}",
                                name=f"w{j}_{s}_{k}")
                nc.sync.dma_start(
                    w[:], d[f"W{j}1"].ap()[s, k * P:(k + 1) * P, :])
                ks.append(w)
            per_s.append(ks)
        wt.append(per_s)
    cst["wt"] = wt
    bias_sb = consts.tile([P, 8 * 16], F32, tag="bias", name="bias")
    nc.sync.dma_start(bias_sb[:], d["bias"].ap())
    cst["bias"] = bias_sb
    bh = consts.tile([OUT, 4], F32, tag="bh", name="bh")
    nc.sync.dma_start(bh[:], d["bh"].ap())
    cst["bh"] = bh
    # broadcast row-biases for the swapped layers: [(t*4+s)] -> [128, H]
    bbc = consts.tile([P, 12 * H], BF16, tag="bbc", name="bbc")
    nc.sync.dma_start(bbc[:], d["bbc"].ap())
    cst["bbc"] = bbc
    return cst


def _emit_body(nc, tc, ctx, d, caps, cst, pools):
    offs, Bp = _offs(caps)
    Cmax = max(caps)
    chunks = _chunks(caps)
    NCH = len(chunks)
    xep, hp, permp, outp, psp = pools

    wf_t, w32_t, wt = cst["wf"], cst["w32"], cst["wt"]
    bias_sb, bh, bbc = cst["bias"], cst["bh"], cst["bbc"]

    xe_t = []
    for j in range(4):
        t = xep.tile([P, Bp], BF16, tag=f"xe{j}", name=f"xe{j}", bufs=2)
        nc.sync.dma_start(t[:], d["xe"].ap()[j, :, :])
        xe_t.append(t)
    tbl = xep.tile([P, 3 * NCH], I32, tag="tbl", name="tbl", bufs=2)
    nc.sync.dma_start(tbl[:], d["tbl"].ap())

    def bias_ap(layer, hh, s):
        col = layer * 16 + hh * 4 + s
        return bias_sb[:, col:col + 1]

    def scol(t, s):
        return slice(offs[s], offs[s] + caps[s])

    # ---------------- layers ----------------
    def first_layer(j, tag):
        """relu(Wf[j].T @ xe_g[j] + b_j0): 4x [128, Bp] bf16, feature-major."""
        outs = []
        for hpair in range(2):
            ps = [[psp.tile([P, H], F32, tag="pt", name="pt")
                   for s in range(M)] for _ in range(2)]
            for hi in range(2):
                hh = hpair * 2 + hi
                for s in range(M):
                    nc.tensor.matmul(
                        ps[hi][s][:, :caps[s]], wf_t[j][:, bass.ts(hh, P)],
                        xe_t[j][:, scol(j, s)], start=True, stop=True)
            for hi in range(2):
                hh = hpair * 2 + hi
                t = hp.tile([P, Bp], BF16, tag=f"{tag}{hh}", name=f"{tag}{hh}")
                for s in range(M):
                    nc.scalar.activation(t[:, scol(j, s)],
                                         ps[hi][s][:, :caps[s]],
                                         RELU, bias=bias_ap(2 * j, hh, s))
                outs.append(t)
        return outs

    def swapped_big(j, z_tiles, t_i):
        """relu(W_j1[slot].T @ z + b), batch-major out -> xsc token tile.

        DVE evacuates PSUM with the row-bias added; Pool applies ReLU
        in-place in SBUF."""
        Kc = KBIG[j]
        xsc = permp.tile([P, NCH, H], BF16, tag="xsc", name="xsc")
        for ch, (s, g0, r) in enumerate(chunks):
            ws = wt[j][s]
            bcol = (t_i * 4 + s) * H
            pb = psp.tile([P, H], F32, tag="pt", name="pt")
            for k in range(Kc):
                nc.tensor.matmul(pb[:r, :], z_tiles[k][:, g0:g0 + r],
                                 ws[k][:],
                                 start=(k == 0), stop=(k == Kc - 1))
            nc.vector.scalar_tensor_tensor(
                xsc[:r, ch, :], pb[:r, :], 0.0, bbc[:r, bcol:bcol + H],
                mybir.AluOpType.bypass, mybir.AluOpType.add)
            nc.vector.tensor_scalar_max(xsc[:r, ch, :], xsc[:r, ch, :], 0.0)
        return xsc

    def transition(t_i, xsc):
        """Scatter chunk tokens into next stage's order; XBAR back."""
        xb = d["xb"][t_i]
        for ch, (s, g0, r) in enumerate(chunks):
            nc.gpsimd.indirect_dma_start(
                xb.ap(),
                bass.IndirectOffsetOnAxis(
                    ap=tbl[:r, t_i * NCH + ch:t_i * NCH + ch + 1], axis=0),
                xsc[:r, ch, :], None)
        zx = permp.tile([P, 4, Bp], BF16, tag="zx", name="zx")
        for k in range(4):
            nc.sync.dma_start(zx[:, k, :], xb.ap()[:, k * P:(k + 1) * P],
                              transpose=True)
        return [zx[:, k, :] for k in range(4)]

    def grouped_big(j, z_tiles, tag):
        """relu(W_j1[slot].T @ z + b): feature-major grouped output."""
        Kc = KBIG[j]
        outs = [hp.tile([P, Bp], BF16, tag=f"{tag}{hh}", name=f"{tag}{hh}")
                for hh in range(4)]
        for s in range(M):
            ws = wt[j][s]
            ps = [psp.tile([P, H], F32, tag="pt", name="pt")
                  for hh in range(4)]
            for k in range(Kc):
                for hh in range(4):
                    nc.tensor.matmul(
                        ps[hh][:, :caps[s]], ws[k][:, bass.ts(hh, P)],
                        z_tiles[k][:, scol(j, s)],
                        start=(k == 0), stop=(k == Kc - 1))
            for hh in range(4):
                nc.scalar.activation(outs[hh][:, scol(j, s)],
                                     ps[hh][:, :caps[s]],
                                     RELU, bias=bias_ap(2 * j + 1, hh, s))
        return outs

    # ---------------- network ----------------
    x = first_layer(0, "h")
    xsc = swapped_big(0, x, 0)
    zx = transition(0, xsc)
    h1 = first_layer(1, "g")
    xsc = swapped_big(1, zx + h1, 1)
    zx = transition(1, xsc)
    h2 = first_layer(2, "h")
    xsc = swapped_big(2, zx + h2, 2)
    zx = transition(2, xsc)
    h3 = first_layer(3, "g")
    x4 = grouped_big(3, zx + h3, "x4")

    # head
    ps = [psp.tile([P, H], F32, tag="pt", name="pt") for s in range(M)]
    for k in range(4):
        for s in range(M):
            nc.tensor.matmul(ps[s][:OUT, :caps[s]], w32_t[s][:, k, :],
                             x4[k][:, scol(3, s)],
                             start=(k == 0), stop=(k == 3))
    out_t = outp.tile([OUT, Bp], F32, tag="outt", name="outt", bufs=2)
    for s in range(M):
        nc.scalar.activation(out_t[:, scol(3, s)], ps[s][:OUT, :caps[s]],
                             COPY)
        nc.vector.tensor_scalar_add(out_t[:, scol(3, s)],
                                    out_t[:, scol(3, s)], bh[:, s:s + 1])
    nc.sync.dma_start(d["out"].ap(), out_t[:])


def build_program(caps, reps: int = 1):
    _, Bp = _offs(caps)
    NCH = len(_chunks(caps))
    nc = bacc.Bacc("TRN2", target_bir_lowering=False, debug=False,
                   enable_asserts=False)
    d = {}
    d["xe"] = nc.dram_tensor("xe", [4, P, Bp], BF16, kind="ExternalInput")
    d["Wf"] = nc.dram_tensor("Wf", [4, P, H], BF16, kind="ExternalInput")
    d["W01"] = nc.dram_tensor("W01", [M, H, H], BF16, kind="ExternalInput")
    d["W11"] = nc.dram_tensor("W11", [M, 2 * H, H], BF16, kind="ExternalInput")
    d["W21"] = nc.dram_tensor("W21", [M, 2 * H, H], BF16, kind="ExternalInput")
    d["W31"] = nc.dram_tensor("W31", [M, 2 * H, H], BF16, kind="ExternalInput")
    d["W32"] = nc.dram_tensor("W32", [M, H, OUT], BF16, kind="ExternalInput")
    d["bias"] = nc.dram_tensor("bias", [P, 8 * 16], F32, kind="ExternalInput")
    d["bh"] = nc.dram_tensor("bh", [OUT, 4], F32, kind="ExternalInput")
    d["bbc"] = nc.dram_tensor("bbc", [P, 12 * H], BF16, kind="ExternalInput")
    d["tbl"] = nc.dram_tensor("tbl", [P, 3 * NCH], I32, kind="ExternalInput")
    d["out"] = nc.dram_tensor("out", [OUT, Bp], F32, kind="ExternalOutput")
    d["xb"] = [nc.dram_tensor(f"xb{i}", [Bp, H], BF16, kind="Internal")
               for i in range(3)]

    with tile.TileContext(nc) as tc, ExitStack() as ctx:
        cst = _emit_consts(nc, tc, ctx, d)
        xep = ctx.enter_context(tc.tile_pool(name="xein", bufs=2))
        hp = ctx.enter_context(tc.tile_pool(name="hacts", bufs=1))
        permp = ctx.enter_context(tc.tile_pool(name="perm", bufs=1))
        outp = ctx.enter_context(tc.tile_pool(name="outs", bufs=2))
        psp = ctx.enter_context(tc.tile_pool(name="psum", bufs=8,
                                             space="PSUM"))
        pools = (xep, hp, permp, outp, psp)
        if reps == 1:
            _emit_body(nc, tc, ctx, d, caps, cst, pools)
        else:
            with tc.For_i(0, reps, 1):
                _emit_body(nc, tc, ctx, d, caps, cst, pools)
    nc.compile()
    return nc


def prep_inputs(inputs):
    iv = np.asarray(inputs["input_val"], dtype=np.float32)
    feats = iv[:, :4 * FEAT]
    oh = iv[:, 4 * FEAT:4 * FEAT + 16]
    idx = [np.argmax(oh[:, 4 * j:4 * j + 4], axis=1) for j in range(4)]

    # per-(core,j) slot order: experts sorted by ascending group size
    perms = np.empty((NCORES, 4, M), np.int64)   # perms[c,j,slot] = expert
    counts = np.empty((NCORES, 4, M), np.int64)  # slot-ordered group sizes
    for c in range(NCORES):
        rc = slice(c * BC, (c + 1) * BC)
        for j in range(4):
            cnt = np.bincount(idx[j][rc], minlength=M)
            perms[c, j] = np.argsort(cnt, kind="stable")
            counts[c, j] = cnt[perms[c, j]]
    rank_max = counts.reshape(-1, M).max(axis=0)
    caps = tuple(int(((v + 31) // 32) * 32) for v in rank_max)
    offs, Bp = _offs(caps)
    chunks = _chunks(caps)
    NCH = len(chunks)

    tobf = lambda a: np.ascontiguousarray(
        np.asarray(a, np.float32).astype(ml_dtypes.bfloat16))

    Wf_full = np.stack(
        [np.asarray(inputs[f"W{j}_0"], np.float32) for j in range(4)])
    # Wf_full[j] is [M, FEAT, H]
    b0 = np.stack(
        [np.asarray(inputs[f"b{j}_0"], np.float32) for j in range(4)])
    b31 = np.asarray(inputs["b3_1"], np.float32)
    bsw = np.stack([np.asarray(inputs[nm], np.float32)
                    for nm in ("b0_1", "b1_1", "b2_1")])  # [3, M, H]
    Wbig = [np.asarray(inputs[f"W{j}_1"], np.float32) for j in range(4)]
    W32_full = np.asarray(inputs["W3_2"], np.float32)
    bh_full = np.asarray(inputs["b3_2"], np.float32)     # [M, OUT]

    in_maps, meta = [], []
    for c in range(NCORES):
        rc = slice(c * BC, (c + 1) * BC)
        pc = perms[c]                       # [4, M] slot -> expert
        inv = np.empty((4, M), np.int64)    # expert -> slot
        for j in range(4):
            inv[j, pc[j]] = np.arange(M)
        sidx = [inv[j][idx[j][rc]] for j in range(4)]   # slot index per row

        orders, slots, padlists = [], [], []
        for j in range(4):
            ij = sidx[j]
            order = np.full(Bp, -1, np.int64)
            slot = np.empty(BC, np.int64)
            pads = []
            for s in range(M):
                rows = np.nonzero(ij == s)[0]
                order[offs[s]:offs[s] + len(rows)] = rows
                slot[rows] = offs[s] + np.arange(len(rows))
                pads.extend(range(offs[s] + len(rows), offs[s] + caps[s]))
            orders.append(order)
            slots.append(slot)
            padlists.append(np.array(pads, np.int64))

        xe = np.zeros((4, P, Bp), np.float32)
        for j in range(4):
            ij = sidx[j]
            fj = feats[rc, FEAT * j:FEAT * (j + 1)]
            for s in range(M):
                rows = np.nonzero(ij == s)[0]
                xe[j, s * FEAT:(s + 1) * FEAT,
                   offs[s]:offs[s] + len(rows)] = fj[rows].T

        tblv = np.full((P, 3 * NCH), Bp + 7, np.int32)   # default: OOB skip
        for t in range(3):
            jp, jn = t, t + 1
            padmap = {int(g): i for i, g in enumerate(padlists[jp])}
            for ch, (s, g0, r) in enumerate(chunks):
                for p in range(r):
                    g = g0 + p
                    src = orders[jp][g]
                    if src >= 0:
                        tblv[p, t * NCH + ch] = slots[jn][src]
                    else:
                        tblv[p, t * NCH + ch] = padlists[jn][padmap[g]]

        # slot-permuted weights/biases for this core
        Wf_c = np.zeros((4, P, H), np.float32)
        bias = np.zeros((P, 8 * 16), np.float32)
        for j in range(4):
            for s in range(M):
                e = pc[j, s]
                Wf_c[j, s * FEAT:(s + 1) * FEAT] = Wf_full[j, e]
                for hh in range(4):
                    bias[:, 2 * j * 16 + hh * 4 + s] = \
                        b0[j, e, hh * P:(hh + 1) * P]
        for s in range(M):
            e = pc[3, s]
            for hh in range(4):
                bias[:, 7 * 16 + hh * 4 + s] = b31[e, hh * P:(hh + 1) * P]
        bbc = np.zeros((P, 12 * H), np.float32)
        for t in range(3):
            for s in range(M):
                e = pc[t, s]
                bbc[:, (t * 4 + s) * H:(t * 4 + s + 1) * H] = \
                    bsw[t, e][None, :]
        in_maps.append({
            "xe": tobf(xe), "tbl": tblv,
            "Wf": tobf(Wf_c), "bias": bias, "bbc": tobf(bbc),
            "W01": tobf(Wbig[0][pc[0]]), "W11": tobf(Wbig[1][pc[1]]),
            "W21": tobf(Wbig[2][pc[2]]), "W31": tobf(Wbig[3][pc[3]]),
            "W32": tobf(W32_full[pc[3]]),
            "bh": np.ascontiguousarray(bh_full[pc[3]].T),
        })
        meta.append(slots[3])
    return caps, in_maps, meta


_CACHE = {}


def kernel(**inputs):
    caps, in_maps, meta = prep_inputs(inputs)
    if ("nc", caps) not in _CACHE:
        _CACHE[("nc", caps)] = build_program(caps)
    nc = _CACHE[("nc", caps)]
    res = bass_utils.run_bass_kernel_spmd(
        nc, in_maps, core_ids=list(range(NCORES)))
    out = np.empty((B, OUT), np.float32)
    for c in range(NCORES):
        o = res.results[c]["out"]
        out[c * BC:(c + 1) * BC] = o[:, meta[c]].T
    return out


if __name__ == "__main__":
    import sys, jax
    import reference
    cpu = jax.local_devices(backend="cpu")[0]
    with jax.default_device(cpu):
        inputs = {k: np.asarray(v) for k, v in reference.setup_inputs().items()}
        exp = np.asarray(reference.reference(**inputs))
    if len(sys.argv) > 1 and sys.argv[1] == "sim":
        from concourse.bass_interp import CoreSim
        caps, in_maps, meta = prep_inputs(inputs)
        print("caps:", caps)
        nc = build_program(caps)
        sim = CoreSim(nc, trace=True)
        for k, v in in_maps[0].items():
            sim.tensor(k)[:] = v
        sim.simulate()
        o = np.asarray(sim.tensor("out"))
        got0 = o[:, meta[0]].T
        exp0 = exp[:BC]
        err = np.abs(got0 - exp0)
        print(f"sim core0 max abs err: {err.max():.3e}  "
              f"rel: {err.max()/np.abs(exp0).max():.3e}")
    else:
        got = kernel(**inputs)
        err = np.abs(got - exp)
        print(f"max abs err: {err.max():.3e}   "
              f"rel: {err.max()/np.abs(exp).max():.3e}")
